# revision 29
# baseline (speedup 1.0000x reference)
"""Trainium2 Bass kernel for nn_Attention (ViT-style attention block).

Reference computation (per batch b, head h):
    qkv  = x @ qkv_weight.T + [q_bias, 0, v_bias]        # [B,N,3C]
    q,k,v split into heads of HD=64;  q *= HD**-0.5
    S    = q @ k.T + relative_position_bias[h]           # [N,N]
    P    = softmax(S, axis=-1)
    O    = P @ v                                         # [N,HD]
    out  = concat_heads(O) @ proj_weight.T + proj_bias   # [B,N,C]

Sharding: pure data-parallel over the batch dim: 16 batches -> 2 per core
across 8 NeuronCores.  Each core gets its own x-shard plus replicated
(host-pre-transposed) weights; outputs are concatenated on the host.

All matmul inputs are bf16 (PSUM accumulation in fp32); tolerance is 2e-2
and measured end-to-end error is ~6e-3.

Device dataflow (per core, B_l=2, T=B_l*N=1154 tokens):
  - qk-pass:  qkT[j, t] (feature-major) = Wqk^T.T @ x^T ; q rows pre-scaled
    by HD**-0.5 on the host (folded into Wq and q_bias).
  - v-pass:   v[t, j] (token-major) = x^T.T @ Wv^T, stored per head with a
    ones column appended: v_aug[t, (h, 0:65)] = [v_h | 1].
  - attention per (head-pair hp, batch): the two heads' S^T matmuls write the
    two halves of one 4-bank PSUM tile (PE row groups 0:63 / 64:127 run
    concurrently); ONE exp on ScalarE covers both heads, then P^T *=
    exp(bias^T) in one tensor_tensor per mt (host precomputes exp of the
    transposed bias; work split between VectorE and GpSimd).
    O'^T for both heads accumulates into the two halves of another PSUM tile
    (row 64 = softmax denominators via the ones column of v_aug).
  - normalization per head-pair: denominator rows collected into an SBUF
    tile via SWDGE, reciprocals on DVE (reciprocal_approx_fast - no ScalarE
    act-table thrash), bf16 partition-broadcast DMA, in-place multiply.
  - proj: out[t, jo] = O^T.T @ Wp^T + proj_bias.

bf16 DMA rule learned on HW: a DMA write run into SBUF must start 4-byte
aligned and may overhang its end by 2 bytes (min 4-byte write granularity)
-- every bf16 destination row here is padded so overhangs land in padding,
never in live data (a 2-byte pad column DMA clobbered its neighbor before).
"""

import numpy as np

B, N, C = 16, 577, 768
H, HD = 12, 64
SCALE = HD ** -0.5
NCORES = 8
BL = B // NCORES           # batches per core (2)
T = BL * N                 # tokens per core (1154)
NT_M = (N + 127) // 128    # m-tiles per batch (5: 4x128 + 65)
CT = C // 128              # 128-contraction tiles over C (6)
NP = N + 1                 # n padded to even (578) for matmul free dims
TP = BL * NP               # padded token rows (1156)

_CACHE = {}


def _chunks(total, limit=512):
    """Bank-aligned matmul free-dim chunks (each <= 512 fp32 = one PSUM bank)."""
    out = []
    pos = 0
    while pos < total:
        n = min(limit, total - pos)
        out.append((pos, n))
        pos += n
    return out


def _build():
    """Trace the Bass/Tile program once. Returns the Bass object."""
    import concourse.bass as bass
    import concourse.tile as tile
    from concourse import bacc, mybir
    from contextlib import ExitStack

    f32 = mybir.dt.float32
    bf16 = mybir.dt.bfloat16
    ALU = mybir.AluOpType
    ACTF = mybir.ActivationFunctionType

    nc = bacc.Bacc("TRN2", target_bir_lowering=False, debug=False)

    # ---- DRAM I/O ----
    xT_d = nc.dram_tensor("xT", [C, T], bf16, kind="ExternalInput").ap()
    wqk_d = nc.dram_tensor("wqkT", [C, 2 * C], bf16, kind="ExternalInput").ap()
    wv_d = nc.dram_tensor("wvT", [C, C], bf16, kind="ExternalInput").ap()
    wp_d = nc.dram_tensor("wpT", [C, C], bf16, kind="ExternalInput").ap()
    qb_d = nc.dram_tensor("qbT", [128, CT], f32, kind="ExternalInput").ap()
    vb_d = nc.dram_tensor("vbB", [128, C], f32, kind="ExternalInput").ap()
    pb_d = nc.dram_tensor("pbB", [128, C], f32, kind="ExternalInput").ap()
    bt_d = nc.dram_tensor("BT", [H, N, N], bf16, kind="ExternalInput").ap()
    ones_d = nc.dram_tensor(
        "ones", [128, BL * NT_M, H, 2], bf16, kind="ExternalInput"
    ).ap()
    out_d = nc.dram_tensor("out", [TP, C], bf16, kind="ExternalOutput").ap()

    def r(x):
        return x

    with tile.TileContext(nc) as tc, ExitStack() as ctx:
        const = ctx.enter_context(tc.tile_pool(name="const", bufs=1))
        persist = ctx.enter_context(tc.tile_pool(name="persist", bufs=1))

        qb_sb = const.tile([128, CT], f32)
        vb_sb = const.tile([128, C], f32)
        pb_sb = const.tile([128, C], f32)

        # Persistent activations
        qk_sb = persist.tile([128, 2 * CT, BL * NP], bf16)      # q^T | k^T
        v_sb = persist.tile([128, BL * NT_M, H, HD + 2], bf16)  # v_aug
        ot_sb = persist.tile([128, CT, BL, NP], bf16)           # O^T (padded)
        dall = persist.tile([4, H // 2 + 1, NP], f32)           # denominators

        # Phase-2/3 SBUF pools are created (and their first tiles allocated)
        # BEFORE phase 1 closes: tiles allocated after phase 1 would alias
        # its xT/wqk/wv SBUF, and their DMAs would inherit a wait on
        # phase-1's last PE reader (observed: rings idle 40us->95us).
        wpp = ctx.enter_context(tc.tile_pool(name="wpp", bufs=1))
        btp = ctx.enter_context(tc.tile_pool(name="btp", bufs=2))
        pp = ctx.enter_context(tc.tile_pool(name="pp", bufs=7))
        sums = ctx.enter_context(tc.tile_pool(name="sums", bufs=3))
        nrm = ctx.enter_context(tc.tile_pool(name="nrm", bufs=2))
        bcst = ctx.enter_context(tc.tile_pool(name="bcst", bufs=4))
        rdp = ctx.enter_context(tc.tile_pool(name="rdp", bufs=2, space="DRAM"))
        oscr = ctx.enter_context(tc.tile_pool(name="oscr", bufs=2))
        outp = ctx.enter_context(tc.tile_pool(name="outp", bufs=3))

        def load_btpair(hp):
            btpair = btp.tile([128, 2, NT_M, NP], bf16)
            for par in range(2):
                h = 2 * hp + par
                ring = nc.sync if par == 0 else nc.scalar
                ring.dma_start(
                    out=btpair[:, par, 0:4, 0:N],
                    in_=bt_d[h, 0:512, :].rearrange("(a p) n -> p a n", p=128),
                )
                ring.dma_start(
                    out=btpair[0:65, par, 4, 0:N], in_=bt_d[h, 512:N, :]
                )
            return btpair

        # ---------------- Phase 1: qkv projections ----------------
        with ExitStack() as p1:
            xp = p1.enter_context(tc.tile_pool(name="xp", bufs=1))
            wvp = p1.enter_context(tc.tile_pool(name="wvp", bufs=1))
            xT_sb = xp.tile([128, CT, T], bf16)
            wv_sb = wvp.tile([128, CT, C], bf16)

            # --- qk-pass (feature-major output) ---
            with ExitStack() as p1b:
                wqkp = p1b.enter_context(tc.tile_pool(name="wqkp", bufs=1))
                wqk_sb = wqkp.tile([128, CT, 2 * C], bf16)
                # one big transfer per tensor (small transfers measured
                # ~100GB/s; >=1MB transfers are needed for full ring rate);
                # xT+wv on the SP ring, wqk on the ACT ring
                nc.sync.dma_start(
                    out=xT_sb[:], in_=xT_d.rearrange("(a p) t -> p a t", p=128)
                )
                nc.scalar.dma_start(
                    out=wqk_sb[:], in_=wqk_d.rearrange("(a p) j -> p a j", p=128)
                )
                nc.sync.dma_start(
                    out=wv_sb[:], in_=wv_d.rearrange("(a p) j -> p a j", p=128)
                )
                nc.sync.dma_start(out=qb_sb[:], in_=qb_d)
                nc.sync.dma_start(out=vb_sb[:], in_=vb_d)
                nc.sync.dma_start(out=pb_sb[:], in_=pb_d)
                nc.gpsimd.dma_start(out=v_sb[:, :, :, HD:HD + 2], in_=ones_d[:])
                # Zero padding columns on a compute engine (NOT a 2-byte DMA,
                # which would clobber the adjacent real column - HW race).
                nc.gpsimd.memset(
                    qk_sb[:].rearrange("p j (b n) -> p j b n", n=NP)[
                        :, :, :, N:N + 1
                    ],
                    0.0,
                )
                nc.gpsimd.memset(ot_sb[:, :, :, N:N + 1], 0.0)
                # Allocate + prefetch proj weights and the first two bias
                # pairs NOW, while phase-1 tiles are live: tiles allocated
                # after phase 1 closes alias its SBUF and their DMAs inherit
                # a wait on phase-1's last PE reader (observed 50us stall).
                wp_sb_l = wpp.tile([128, CT, C], bf16)
                nc.scalar.dma_start(
                    out=wp_sb_l[:], in_=wp_d.rearrange("(a p) j -> p a j", p=128)
                )
                bt_prefetch = [load_btpair(0), load_btpair(1)]
                qkps = p1b.enter_context(
                    tc.tile_pool(name="qkps", bufs=2, space="PSUM")
                )
                vb_v = vb_sb[:].rearrange("p (h d) -> p h d", d=HD)

                def qk_chain(jt):
                    ps = qkps.tile([128, T], f32, tag="p1")
                    for ct in range(CT):
                        for (t0, tn) in _chunks(T):
                            nc.tensor.matmul(
                                ps[:, t0:t0 + tn],
                                lhsT=r(wqk_sb[:, ct, jt * 128:(jt + 1) * 128]),
                                rhs=r(xT_sb[:, ct, t0:t0 + tn]),
                                start=(ct == 0),
                                stop=(ct == CT - 1),
                            )
                    qk_dst = qk_sb[:, jt, :].rearrange(
                        "p (b n) -> p b n", n=NP
                    )[:, :, 0:N]
                    ps_v = ps[:].rearrange("p (b n) -> p b n", n=N)
                    if jt < CT:
                        # q: add host-pre-scaled bias (per feature = partition)
                        nc.vector.tensor_scalar(
                            out=qk_dst,
                            in0=ps_v,
                            scalar1=qb_sb[:, jt:jt + 1],
                            scalar2=None,
                            op0=ALU.add,
                        )
                    else:
                        nc.vector.tensor_copy(qk_dst, ps_v)

                def v_chain(b):
                    for mt in range(NT_M):
                        mp = min(128, N - mt * 128)
                        t0 = b * N + mt * 128
                        idx = b * NT_M + mt
                        ps = qkps.tile([128, T], f32, tag="p1")
                        for ct in range(CT):
                            for (j0, jn) in _chunks(C):
                                nc.tensor.matmul(
                                    ps[0:mp, j0:j0 + jn],
                                    lhsT=r(xT_sb[:, ct, t0:t0 + mp]),
                                    rhs=r(wv_sb[:, ct, j0:j0 + jn]),
                                    start=(ct == 0),
                                    stop=(ct == CT - 1),
                                )
                        nc.vector.tensor_add(
                            v_sb[0:mp, idx, :, 0:HD],
                            ps[0:mp, 0:C].rearrange("p (h d) -> p h d", d=HD),
                            vb_v[0:mp],
                        )

                # jt chains consume wqk halves in DMA arrival order; the
                # v-pass runs last (wv lands behind xT on the sync ring).
                for jt in range(2 * CT):
                    qk_chain(jt)
                v_chain(0)
                v_chain(1)


        # ---------------- Phase 2: attention ----------------
        with ExitStack() as p2:
            # one PSUM pool: S-pair tiles and O-pair tiles share it.
            # [128, 2, 1024] f32 = 4 banks; bufs=2 = all 8 banks.
            aps = p2.enter_context(tc.tile_pool(name="aps", bufs=2, space="PSUM"))

            def s_phase(hp, b, btpair):
                """Paired S^T matmuls for both heads of hp (PE row groups
                0:63/64:127 run concurrently) into the two halves of one PSUM
                tile; ONE exp for both heads per mt; one bias-mult per mt.
                Returns the 5 P-pair tiles."""
                qTs, kTs = [], []
                for par in range(2):
                    base = par * 64
                    qTs.append(qk_sb[base:base + 64, hp, b * NP:(b + 1) * NP])
                    kTs.append(qk_sb[base:base + 64, CT + hp, b * NP:(b + 1) * NP])
                pts = []
                for mt in range(NT_M):
                    mp = min(128, N - mt * 128)
                    sp = aps.tile([128, 2, 1024], f32, tag="aps")
                    for par in range(2):
                        for (n0, nn) in _chunks(NP):
                            nc.tensor.matmul(
                                sp[0:mp, par, n0:n0 + nn],
                                lhsT=r(kTs[par][:, mt * 128:mt * 128 + mp]),
                                rhs=r(qTs[par][:, n0:n0 + nn]),
                                start=True,
                                stop=True,
                            )
                    pt = pp.tile([128, 2, NP], bf16, tag="ptile")
                    nc.scalar.activation(
                        pt[0:mp, :, :], sp[0:mp, :, 0:NP], ACTF.Exp
                    )
                    eng = nc.gpsimd if mt == 1 else nc.vector
                    eng.tensor_mul(
                        pt[0:mp, :, 0:N], pt[0:mp, :, 0:N],
                        btpair[0:mp, :, mt, 0:N],
                    )
                    pts.append(pt)
                return pts

            def o_phase(hp, b, pts, drow=None, dcol=None):
                """O'^T for both heads into the two halves of one PSUM tile;
                row 64 = softmax denominators (ones column of v_aug)."""
                ou = aps.tile([128, 2, 1024], f32, tag="aps")
                for mt in range(NT_M):
                    mp = min(128, N - mt * 128)
                    for par in range(2):
                        h = 2 * hp + par
                        for (n0, nn) in _chunks(NP):
                            nc.tensor.matmul(
                                ou[0:HD + 2, par, n0:n0 + nn],
                                lhsT=r(v_sb[0:mp, b * NT_M + mt, h, :]),
                                rhs=r(pts[mt][0:mp, par, n0:n0 + nn]),
                                start=(mt == 0),
                                stop=(mt == NT_M - 1),
                            )
                # both heads' denominator rows -> dall (SWDGE, SBUF->SBUF)
                sm = sums.tile([HD + 1, 2, N], f32, tag="sm")
                nc.scalar.copy(sm[HD:HD + 1, :, :], ou[HD:HD + 1, :, 0:N])
                if drow is None:
                    drow, dcol = 2 * b, hp
                nc.gpsimd.dma_start(
                    out=dall[drow:drow + 2, dcol, 0:N], in_=sm[HD:HD + 1, :, :]
                )
                nc.vector.tensor_copy(
                    ot_sb[0:64, hp, b, 0:N], ou[0:64, 0, 0:N]
                )
                sc = oscr.tile([64, N], bf16)
                nc.vector.tensor_copy(sc[:], ou[0:64, 1, 0:N])
                nc.sync.dma_start(
                    out=ot_sb[64:128, hp, b, 0:N], in_=sc[:]
                )

            def normalize(hp, dcol, ni, units):
                """Reciprocals for `units` = list of (b, par) stored at
                dall[0:ni, dcol] on DVE, bf16 broadcast, in-place multiply."""
                rall = nrm.tile([4, NP], f32, tag="rall")
                nc.vector.reciprocal_approx_fast(
                    rall[0:ni, 0:N], dall[0:ni, dcol, 0:N]
                )
                rbf = nrm.tile([4, NP], bf16, tag="rbf")
                nc.vector.tensor_copy(rbf[0:ni, 0:N], rall[0:ni, 0:N])
                # partition-broadcast DMA needs a DRAM source: tiny bounce
                rdr = rdp.tile([4, NP], bf16, tag="rdr")
                nc.gpsimd.dma_start(out=rdr[0:ni, 0:N], in_=rbf[0:ni, 0:N])
                for i, (b, par) in enumerate(units):
                    base = par * 64
                    bc = bcst.tile([128, NP], bf16, tag="bc")
                    nc.gpsimd.dma_start(
                        out=bc[base:base + 64, 0:N],
                        in_=rdr[i:i + 1, 0:N].broadcast_to([64, N]),
                    )
                    eng = nc.vector if par == 0 else nc.gpsimd
                    eng.tensor_mul(
                        ot_sb[base:base + 64, hp, b, 0:N],
                        ot_sb[base:base + 64, hp, b, 0:N],
                        bc[base:base + 64, 0:N],
                    )

            bts = {0: bt_prefetch[0], 1: bt_prefetch[1]}
            for hp in range(H // 2):
                # issue the NEXT pair's load now: its pool buffer (freed at
                # hp-1's end) is available, so the transfer overlaps all of
                # this head-pair's compute instead of starting at hp+1.
                if hp + 1 < H // 2 and hp + 1 not in bts:
                    bts[hp + 1] = load_btpair(hp + 1)
                btpair = bts.pop(hp)
                last = hp == H // 2 - 1
                for b in range(BL):
                    pts = s_phase(hp, b, btpair)
                    if last:
                        # final head-pair: each batch gets its own dall column
                        # (engine base-partition must be 0) and normalizes
                        # immediately so proj's dependency resolves early
                        o_phase(hp, b, pts, drow=0, dcol=hp + b)
                        normalize(hp, hp + b, 2, [(b, 0), (b, 1)])
                    else:
                        o_phase(hp, b, pts)
                if not last:
                    normalize(hp, hp, 4, [(0, 0), (0, 1), (1, 0), (1, 1)])

        # ---------------- Phase 3: output projection ----------------
        with ExitStack() as p3:
            pps = p3.enter_context(tc.tile_pool(name="pps", bufs=2, space="PSUM"))
            ntt = (TP + 127) // 128
            ot_flat = ot_sb[:].rearrange("p c b n -> p c (b n)")
            for tt in range(ntt):
                tp = min(128, TP - tt * 128)
                ps = pps.tile([128, C], f32)
                for ct in range(CT):
                    for (j0, jn) in _chunks(C):
                        nc.tensor.matmul(
                            ps[0:tp, j0:j0 + jn],
                            lhsT=r(ot_flat[:, ct, tt * 128:tt * 128 + tp]),
                            rhs=r(wp_sb_l[:, ct, j0:j0 + jn]),
                            start=(ct == 0),
                            stop=(ct == CT - 1),
                        )
                os = outp.tile([128, C], bf16)
                nc.vector.tensor_add(os[0:tp, :], ps[0:tp, :], pb_sb[0:tp, :])
                ring = nc.sync if tt % 2 == 0 else nc.scalar
                ring.dma_start(
                    out=out_d[tt * 128:tt * 128 + tp, :], in_=os[0:tp, :]
                )

    nc.compile()
    return nc


def _get_nc():
    if "nc" not in _CACHE:
        _CACHE["nc"] = _build()
    return _CACHE["nc"]


def _prep_inputs(x, relative_position_bias, qkv_weight, q_bias, v_bias,
                 proj_weight, proj_bias):
    """Host-side layout prep + per-core sharding. Returns list of in_maps."""
    import ml_dtypes

    f = np.float32
    bf = ml_dtypes.bfloat16
    x = np.asarray(x, f)
    bias = np.asarray(relative_position_bias, f)
    w = np.asarray(qkv_weight, f)
    qb = np.asarray(q_bias, f)
    vb = np.asarray(v_bias, f)
    wp = np.asarray(proj_weight, f)
    pb = np.asarray(proj_bias, f)

    wq_s = w[0:C] * f(SCALE)            # fold q scaling into weights/bias
    qb_s = qb * f(SCALE)
    wqkT = np.ascontiguousarray(np.concatenate([wq_s, w[C:2 * C]], 0).T.astype(bf))
    wvT = np.ascontiguousarray(w[2 * C:].T.astype(bf))
    wpT = np.ascontiguousarray(wp.T.astype(bf))
    qbT = np.ascontiguousarray(qb_s.reshape(CT, 128).T)
    vbB = np.ascontiguousarray(np.broadcast_to(vb, (128, C)))
    pbB = np.ascontiguousarray(np.broadcast_to(pb, (128, C)))
    BT = np.ascontiguousarray(
        np.exp(bias.transpose(0, 2, 1), dtype=np.float32).astype(bf)
    )

    ones = np.zeros((128, BL * NT_M, H, 2), dtype=bf)
    ones[:, :, :, 0] = 1.0
    shared = dict(wqkT=wqkT, wvT=wvT, wpT=wpT, qbT=qbT, vbB=vbB, pbB=pbB, BT=BT,
                  ones=ones)
    in_maps = []
    for c in range(NCORES):
        xs = x[c * BL:(c + 1) * BL].reshape(T, C)
        in_maps.append(dict(shared, xT=np.ascontiguousarray(xs.T.astype(bf))))
    return in_maps


def kernel(x, relative_position_bias, qkv_weight, q_bias, v_bias,
           proj_weight, proj_bias):
    from concourse import bass_utils

    in_maps = _prep_inputs(x, relative_position_bias, qkv_weight, q_bias,
                           v_bias, proj_weight, proj_bias)
    nc = _get_nc()
    res = bass_utils.run_bass_kernel_spmd(nc, in_maps, core_ids=list(range(NCORES)))
    out = np.concatenate(
        [res.results[c]["out"].reshape(BL, NP, C)[:, :N, :] for c in range(NCORES)],
        axis=0,
    )
    return out.astype(np.float32)


# revision 30
# speedup vs baseline: 1.0245x; 1.0245x over previous
"""Trainium2 Bass kernel for nn_Attention (ViT-style attention block).

Reference computation (per batch b, head h):
    qkv  = x @ qkv_weight.T + [q_bias, 0, v_bias]        # [B,N,3C]
    q,k,v split into heads of HD=64;  q *= HD**-0.5
    S    = q @ k.T + relative_position_bias[h]           # [N,N]
    P    = softmax(S, axis=-1)
    O    = P @ v                                         # [N,HD]
    out  = concat_heads(O) @ proj_weight.T + proj_bias   # [B,N,C]

Sharding: pure data-parallel over the batch dim: 16 batches -> 2 per core
across 8 NeuronCores.  Each core gets its own x-shard plus replicated
(host-pre-transposed) weights; outputs are concatenated on the host.

All matmul inputs are bf16 (PSUM accumulation in fp32); tolerance is 2e-2
and measured end-to-end error is ~6e-3.

Device dataflow (per core, B_l=2, T=B_l*N=1154 tokens):
  - qk-pass:  qkT[j, t] (feature-major) = Wqk^T.T @ x^T ; q rows pre-scaled
    by HD**-0.5 on the host (folded into Wq and q_bias).
  - v-pass:   v[t, j] (token-major) = x^T.T @ Wv^T, stored per head with a
    ones column appended: v_aug[t, (h, 0:65)] = [v_h | 1].
  - attention per (head-pair hp, batch): the two heads' S^T matmuls write the
    two halves of one 4-bank PSUM tile (PE row groups 0:63 / 64:127 run
    concurrently); ONE exp on ScalarE covers both heads, then P^T *=
    exp(bias^T) in one tensor_tensor per mt (host precomputes exp of the
    transposed bias; work split between VectorE and GpSimd).
    O'^T for both heads accumulates into the two halves of another PSUM tile
    (row 64 = softmax denominators via the ones column of v_aug).
  - normalization per head-pair: denominator rows collected into an SBUF
    tile via SWDGE, reciprocals on DVE (reciprocal_approx_fast - no ScalarE
    act-table thrash), bf16 partition-broadcast DMA, in-place multiply.
  - proj: out[t, jo] = O^T.T @ Wp^T + proj_bias.

bf16 DMA rule learned on HW: a DMA write run into SBUF must start 4-byte
aligned and may overhang its end by 2 bytes (min 4-byte write granularity)
-- every bf16 destination row here is padded so overhangs land in padding,
never in live data (a 2-byte pad column DMA clobbered its neighbor before).
"""

import numpy as np

B, N, C = 16, 577, 768
H, HD = 12, 64
SCALE = HD ** -0.5
NCORES = 8
BL = B // NCORES           # batches per core (2)
T = BL * N                 # tokens per core (1154)
NT_M = (N + 127) // 128    # m-tiles per batch (5: 4x128 + 65)
CT = C // 128              # 128-contraction tiles over C (6)
NP = N + 1                 # n padded to even (578) for matmul free dims
TP = BL * NP               # padded token rows (1156)

_CACHE = {}


def _chunks(total, limit=512):
    """Bank-aligned matmul free-dim chunks (each <= 512 fp32 = one PSUM bank)."""
    out = []
    pos = 0
    while pos < total:
        n = min(limit, total - pos)
        out.append((pos, n))
        pos += n
    return out


def _build():
    """Trace the Bass/Tile program once. Returns the Bass object."""
    import concourse.bass as bass
    import concourse.tile as tile
    from concourse import bacc, mybir
    from contextlib import ExitStack

    f32 = mybir.dt.float32
    bf16 = mybir.dt.bfloat16
    ALU = mybir.AluOpType
    ACTF = mybir.ActivationFunctionType

    nc = bacc.Bacc("TRN2", target_bir_lowering=False, debug=False)

    # ---- DRAM I/O ----
    xT_d = nc.dram_tensor("xT", [C, T], bf16, kind="ExternalInput").ap()
    wqk_d = nc.dram_tensor("wqkT", [C, 2 * C], bf16, kind="ExternalInput").ap()
    wv_d = nc.dram_tensor("wvT", [C, C], bf16, kind="ExternalInput").ap()
    wp_d = nc.dram_tensor("wpT", [C, C], bf16, kind="ExternalInput").ap()
    qb_d = nc.dram_tensor("qbT", [128, CT], f32, kind="ExternalInput").ap()
    vb_d = nc.dram_tensor("vbB", [128, C], f32, kind="ExternalInput").ap()
    pb_d = nc.dram_tensor("pbB", [128, C], f32, kind="ExternalInput").ap()
    bt_d = nc.dram_tensor("BT", [H, N, N], bf16, kind="ExternalInput").ap()
    ones_d = nc.dram_tensor(
        "ones", [128, BL * NT_M, H, 2], bf16, kind="ExternalInput"
    ).ap()
    out_d = nc.dram_tensor("out", [TP, C], bf16, kind="ExternalOutput").ap()

    def r(x):
        return x

    with tile.TileContext(nc) as tc, ExitStack() as ctx:
        const = ctx.enter_context(tc.tile_pool(name="const", bufs=1))
        persist = ctx.enter_context(tc.tile_pool(name="persist", bufs=1))

        qb_sb = const.tile([128, CT], f32)
        vb_sb = const.tile([128, C], f32)
        pb_sb = const.tile([128, C], f32)

        # Persistent activations
        qk_sb = persist.tile([128, 2 * CT, BL * NP], bf16)      # q^T | k^T
        v_sb = persist.tile([128, BL * NT_M, H, HD + 2], bf16)  # v_aug
        ot_sb = persist.tile([128, CT, BL, NP], bf16)           # O^T (padded)
        dall = persist.tile([4, H // 2 + 1, NP], f32)           # denominators

        # Phase-2/3 SBUF pools are created (and their first tiles allocated)
        # BEFORE phase 1 closes: tiles allocated after phase 1 would alias
        # its xT/wqk/wv SBUF, and their DMAs would inherit a wait on
        # phase-1's last PE reader (observed: rings idle 40us->95us).
        wpp = ctx.enter_context(tc.tile_pool(name="wpp", bufs=1))
        btp = ctx.enter_context(tc.tile_pool(name="btp", bufs=2))
        pp = ctx.enter_context(tc.tile_pool(name="pp", bufs=7))
        sums = ctx.enter_context(tc.tile_pool(name="sums", bufs=3))
        nrm = ctx.enter_context(tc.tile_pool(name="nrm", bufs=2))
        bcst = ctx.enter_context(tc.tile_pool(name="bcst", bufs=4))
        rdp = ctx.enter_context(tc.tile_pool(name="rdp", bufs=2, space="DRAM"))
        oscr = ctx.enter_context(tc.tile_pool(name="oscr", bufs=2))
        outp = ctx.enter_context(tc.tile_pool(name="outp", bufs=3))

        def load_btpair(hp):
            btpair = btp.tile([128, 2, NT_M, NP], bf16)
            for par in range(2):
                h = 2 * hp + par
                ring = nc.sync if par == 0 else nc.scalar
                ring.dma_start(
                    out=btpair[:, par, 0:4, 0:N],
                    in_=bt_d[h, 0:512, :].rearrange("(a p) n -> p a n", p=128),
                )
                ring.dma_start(
                    out=btpair[0:65, par, 4, 0:N], in_=bt_d[h, 512:N, :]
                )
            return btpair

        # ---------------- Phase 1: qkv projections ----------------
        with ExitStack() as p1:
            xp = p1.enter_context(tc.tile_pool(name="xp", bufs=1))
            wvp = p1.enter_context(tc.tile_pool(name="wvp", bufs=1))
            xT_sb = xp.tile([128, CT, T], bf16)
            wv_sb = wvp.tile([128, CT, C], bf16)

            # --- qk-pass (feature-major output) ---
            with ExitStack() as p1b:
                wqkp = p1b.enter_context(tc.tile_pool(name="wqkp", bufs=1))
                wqk_sb = wqkp.tile([128, CT, 2 * C], bf16)
                # per-ct transfers: multiple outstanding medium transfers
                # outperform one large one (measured); xT+wv on the SP ring,
                # wqk on the ACT ring
                for ct in range(CT):
                    nc.scalar.dma_start(
                        out=wqk_sb[:, ct, :],
                        in_=wqk_d[ct * 128:(ct + 1) * 128, :],
                    )
                    nc.sync.dma_start(
                        out=xT_sb[:, ct, :],
                        in_=xT_d[ct * 128:(ct + 1) * 128, :],
                    )
                for ct in range(CT):
                    nc.sync.dma_start(
                        out=wv_sb[:, ct, :],
                        in_=wv_d[ct * 128:(ct + 1) * 128, :],
                    )
                nc.sync.dma_start(out=qb_sb[:], in_=qb_d)
                nc.sync.dma_start(out=vb_sb[:], in_=vb_d)
                nc.sync.dma_start(out=pb_sb[:], in_=pb_d)
                nc.gpsimd.dma_start(out=v_sb[:, :, :, HD:HD + 2], in_=ones_d[:])
                # Zero padding columns on a compute engine (NOT a 2-byte DMA,
                # which would clobber the adjacent real column - HW race).
                nc.gpsimd.memset(
                    qk_sb[:].rearrange("p j (b n) -> p j b n", n=NP)[
                        :, :, :, N:N + 1
                    ],
                    0.0,
                )
                nc.gpsimd.memset(ot_sb[:, :, :, N:N + 1], 0.0)
                # Allocate + prefetch proj weights and the first two bias
                # pairs NOW, while phase-1 tiles are live: tiles allocated
                # after phase 1 closes alias its SBUF and their DMAs inherit
                # a wait on phase-1's last PE reader (observed 50us stall).
                wp_sb_l = wpp.tile([128, CT, C], bf16)
                nc.scalar.dma_start(
                    out=wp_sb_l[:], in_=wp_d.rearrange("(a p) j -> p a j", p=128)
                )
                bt_prefetch = [load_btpair(0), load_btpair(1)]
                qkps = p1b.enter_context(
                    tc.tile_pool(name="qkps", bufs=2, space="PSUM")
                )
                vb_v = vb_sb[:].rearrange("p (h d) -> p h d", d=HD)

                def qk_chain(jt):
                    ps = qkps.tile([128, T], f32, tag="p1")
                    for ct in range(CT):
                        for (t0, tn) in _chunks(T):
                            nc.tensor.matmul(
                                ps[:, t0:t0 + tn],
                                lhsT=r(wqk_sb[:, ct, jt * 128:(jt + 1) * 128]),
                                rhs=r(xT_sb[:, ct, t0:t0 + tn]),
                                start=(ct == 0),
                                stop=(ct == CT - 1),
                            )
                    qk_dst = qk_sb[:, jt, :].rearrange(
                        "p (b n) -> p b n", n=NP
                    )[:, :, 0:N]
                    ps_v = ps[:].rearrange("p (b n) -> p b n", n=N)
                    if jt < CT:
                        # q: add host-pre-scaled bias (per feature = partition)
                        nc.vector.tensor_scalar(
                            out=qk_dst,
                            in0=ps_v,
                            scalar1=qb_sb[:, jt:jt + 1],
                            scalar2=None,
                            op0=ALU.add,
                        )
                    else:
                        nc.vector.tensor_copy(qk_dst, ps_v)

                def v_chain(b):
                    for mt in range(NT_M):
                        mp = min(128, N - mt * 128)
                        t0 = b * N + mt * 128
                        idx = b * NT_M + mt
                        ps = qkps.tile([128, T], f32, tag="p1")
                        for ct in range(CT):
                            for (j0, jn) in _chunks(C):
                                nc.tensor.matmul(
                                    ps[0:mp, j0:j0 + jn],
                                    lhsT=r(xT_sb[:, ct, t0:t0 + mp]),
                                    rhs=r(wv_sb[:, ct, j0:j0 + jn]),
                                    start=(ct == 0),
                                    stop=(ct == CT - 1),
                                )
                        nc.vector.tensor_add(
                            v_sb[0:mp, idx, :, 0:HD],
                            ps[0:mp, 0:C].rearrange("p (h d) -> p h d", d=HD),
                            vb_v[0:mp],
                        )

                # jt chains consume wqk halves in DMA arrival order; the
                # v-pass runs last (wv lands behind xT on the sync ring).
                for jt in range(2 * CT):
                    qk_chain(jt)
                v_chain(0)
                v_chain(1)


        # ---------------- Phase 2: attention ----------------
        with ExitStack() as p2:
            # one PSUM pool: S-pair tiles and O-pair tiles share it.
            # [128, 2, 1024] f32 = 4 banks; bufs=2 = all 8 banks.
            aps = p2.enter_context(tc.tile_pool(name="aps", bufs=2, space="PSUM"))

            def s_phase(hp, b, btpair):
                """Paired S^T matmuls for both heads of hp (PE row groups
                0:63/64:127 run concurrently) into the two halves of one PSUM
                tile; ONE exp for both heads per mt; one bias-mult per mt.
                Returns the 5 P-pair tiles."""
                qTs, kTs = [], []
                for par in range(2):
                    base = par * 64
                    qTs.append(qk_sb[base:base + 64, hp, b * NP:(b + 1) * NP])
                    kTs.append(qk_sb[base:base + 64, CT + hp, b * NP:(b + 1) * NP])
                pts = []
                for mt in range(NT_M):
                    mp = min(128, N - mt * 128)
                    sp = aps.tile([128, 2, 1024], f32, tag="aps")
                    for par in range(2):
                        for (n0, nn) in _chunks(NP):
                            nc.tensor.matmul(
                                sp[0:mp, par, n0:n0 + nn],
                                lhsT=r(kTs[par][:, mt * 128:mt * 128 + mp]),
                                rhs=r(qTs[par][:, n0:n0 + nn]),
                                start=True,
                                stop=True,
                            )
                    pt = pp.tile([128, 2, NP], bf16, tag="ptile")
                    nc.scalar.activation(
                        pt[0:mp, :, :], sp[0:mp, :, 0:NP], ACTF.Exp
                    )
                    eng = nc.gpsimd if mt == 1 else nc.vector
                    eng.tensor_mul(
                        pt[0:mp, :, 0:N], pt[0:mp, :, 0:N],
                        btpair[0:mp, :, mt, 0:N],
                    )
                    pts.append(pt)
                return pts

            def o_phase(hp, b, pts, drow=None, dcol=None):
                """O'^T for both heads into the two halves of one PSUM tile;
                row 64 = softmax denominators (ones column of v_aug)."""
                ou = aps.tile([128, 2, 1024], f32, tag="aps")
                for mt in range(NT_M):
                    mp = min(128, N - mt * 128)
                    for par in range(2):
                        h = 2 * hp + par
                        for (n0, nn) in _chunks(NP):
                            nc.tensor.matmul(
                                ou[0:HD + 2, par, n0:n0 + nn],
                                lhsT=r(v_sb[0:mp, b * NT_M + mt, h, :]),
                                rhs=r(pts[mt][0:mp, par, n0:n0 + nn]),
                                start=(mt == 0),
                                stop=(mt == NT_M - 1),
                            )
                # both heads' denominator rows -> dall (SWDGE, SBUF->SBUF)
                sm = sums.tile([HD + 1, 2, N], f32, tag="sm")
                nc.scalar.copy(sm[HD:HD + 1, :, :], ou[HD:HD + 1, :, 0:N])
                if drow is None:
                    drow, dcol = 2 * b, hp
                nc.gpsimd.dma_start(
                    out=dall[drow:drow + 2, dcol, 0:N], in_=sm[HD:HD + 1, :, :]
                )
                nc.vector.tensor_copy(
                    ot_sb[0:64, hp, b, 0:N], ou[0:64, 0, 0:N]
                )
                sc = oscr.tile([64, N], bf16)
                nc.vector.tensor_copy(sc[:], ou[0:64, 1, 0:N])
                nc.sync.dma_start(
                    out=ot_sb[64:128, hp, b, 0:N], in_=sc[:]
                )

            def normalize(hp, dcol, ni, units):
                """Reciprocals for `units` = list of (b, par) stored at
                dall[0:ni, dcol] on DVE, bf16 broadcast, in-place multiply."""
                rall = nrm.tile([4, NP], f32, tag="rall")
                nc.vector.reciprocal_approx_fast(
                    rall[0:ni, 0:N], dall[0:ni, dcol, 0:N]
                )
                rbf = nrm.tile([4, NP], bf16, tag="rbf")
                nc.vector.tensor_copy(rbf[0:ni, 0:N], rall[0:ni, 0:N])
                # partition-broadcast DMA needs a DRAM source: tiny bounce
                rdr = rdp.tile([4, NP], bf16, tag="rdr")
                nc.gpsimd.dma_start(out=rdr[0:ni, 0:N], in_=rbf[0:ni, 0:N])
                for i, (b, par) in enumerate(units):
                    base = par * 64
                    bc = bcst.tile([128, NP], bf16, tag="bc")
                    nc.gpsimd.dma_start(
                        out=bc[base:base + 64, 0:N],
                        in_=rdr[i:i + 1, 0:N].broadcast_to([64, N]),
                    )
                    eng = nc.vector if par == 0 else nc.gpsimd
                    eng.tensor_mul(
                        ot_sb[base:base + 64, hp, b, 0:N],
                        ot_sb[base:base + 64, hp, b, 0:N],
                        bc[base:base + 64, 0:N],
                    )

            bts = {0: bt_prefetch[0], 1: bt_prefetch[1]}
            for hp in range(H // 2):
                # issue the NEXT pair's load now: its pool buffer (freed at
                # hp-1's end) is available, so the transfer overlaps all of
                # this head-pair's compute instead of starting at hp+1.
                if hp + 1 < H // 2 and hp + 1 not in bts:
                    bts[hp + 1] = load_btpair(hp + 1)
                btpair = bts.pop(hp)
                last = hp == H // 2 - 1
                for b in range(BL):
                    pts = s_phase(hp, b, btpair)
                    if last:
                        # final head-pair: each batch gets its own dall column
                        # (engine base-partition must be 0) and normalizes
                        # immediately so proj's dependency resolves early
                        o_phase(hp, b, pts, drow=0, dcol=hp + b)
                        normalize(hp, hp + b, 2, [(b, 0), (b, 1)])
                    else:
                        o_phase(hp, b, pts)
                if not last:
                    normalize(hp, hp, 4, [(0, 0), (0, 1), (1, 0), (1, 1)])

        # ---------------- Phase 3: output projection ----------------
        with ExitStack() as p3:
            pps = p3.enter_context(tc.tile_pool(name="pps", bufs=2, space="PSUM"))
            ntt = (TP + 127) // 128
            ot_flat = ot_sb[:].rearrange("p c b n -> p c (b n)")
            for tt in range(ntt):
                tp = min(128, TP - tt * 128)
                ps = pps.tile([128, C], f32)
                for ct in range(CT):
                    for (j0, jn) in _chunks(C):
                        nc.tensor.matmul(
                            ps[0:tp, j0:j0 + jn],
                            lhsT=r(ot_flat[:, ct, tt * 128:tt * 128 + tp]),
                            rhs=r(wp_sb_l[:, ct, j0:j0 + jn]),
                            start=(ct == 0),
                            stop=(ct == CT - 1),
                        )
                os = outp.tile([128, C], bf16)
                nc.vector.tensor_add(os[0:tp, :], ps[0:tp, :], pb_sb[0:tp, :])
                ring = nc.sync if tt % 2 == 0 else nc.scalar
                ring.dma_start(
                    out=out_d[tt * 128:tt * 128 + tp, :], in_=os[0:tp, :]
                )

    nc.compile()
    return nc


def _get_nc():
    if "nc" not in _CACHE:
        _CACHE["nc"] = _build()
    return _CACHE["nc"]


def _prep_inputs(x, relative_position_bias, qkv_weight, q_bias, v_bias,
                 proj_weight, proj_bias):
    """Host-side layout prep + per-core sharding. Returns list of in_maps."""
    import ml_dtypes

    f = np.float32
    bf = ml_dtypes.bfloat16
    x = np.asarray(x, f)
    bias = np.asarray(relative_position_bias, f)
    w = np.asarray(qkv_weight, f)
    qb = np.asarray(q_bias, f)
    vb = np.asarray(v_bias, f)
    wp = np.asarray(proj_weight, f)
    pb = np.asarray(proj_bias, f)

    wq_s = w[0:C] * f(SCALE)            # fold q scaling into weights/bias
    qb_s = qb * f(SCALE)
    wqkT = np.ascontiguousarray(np.concatenate([wq_s, w[C:2 * C]], 0).T.astype(bf))
    wvT = np.ascontiguousarray(w[2 * C:].T.astype(bf))
    wpT = np.ascontiguousarray(wp.T.astype(bf))
    qbT = np.ascontiguousarray(qb_s.reshape(CT, 128).T)
    vbB = np.ascontiguousarray(np.broadcast_to(vb, (128, C)))
    pbB = np.ascontiguousarray(np.broadcast_to(pb, (128, C)))
    BT = np.ascontiguousarray(
        np.exp(bias.transpose(0, 2, 1), dtype=np.float32).astype(bf)
    )

    ones = np.zeros((128, BL * NT_M, H, 2), dtype=bf)
    ones[:, :, :, 0] = 1.0
    shared = dict(wqkT=wqkT, wvT=wvT, wpT=wpT, qbT=qbT, vbB=vbB, pbB=pbB, BT=BT,
                  ones=ones)
    in_maps = []
    for c in range(NCORES):
        xs = x[c * BL:(c + 1) * BL].reshape(T, C)
        in_maps.append(dict(shared, xT=np.ascontiguousarray(xs.T.astype(bf))))
    return in_maps


def kernel(x, relative_position_bias, qkv_weight, q_bias, v_bias,
           proj_weight, proj_bias):
    from concourse import bass_utils

    in_maps = _prep_inputs(x, relative_position_bias, qkv_weight, q_bias,
                           v_bias, proj_weight, proj_bias)
    nc = _get_nc()
    res = bass_utils.run_bass_kernel_spmd(nc, in_maps, core_ids=list(range(NCORES)))
    out = np.concatenate(
        [res.results[c]["out"].reshape(BL, NP, C)[:, :N, :] for c in range(NCORES)],
        axis=0,
    )
    return out.astype(np.float32)
